# revision 1
# baseline (speedup 1.0000x reference)
"""GCN (2x GCNConv + linear head) on 8 TRN2 NeuronCores.

Strategy (graph-parallel by target node):
- Nodes are sharded across 8 cores (6250 real + padding = 6400 rows/core,
  table numbering: table_row = core*6400 + local).
- Layer tables H = dis * (x @ W) live in DRAM, rows are 512B (128 f32) so
  dma_gather fetches one edge-message per descriptor at line rate.
- Edges are grouped per (core, 256-target window, lo/hi source range) --
  the lo/hi split works around dma_gather's int16 index range.  Each
  128-edge block becomes one fp32r matmul: PSUM[feat, tgt] +=
  gathered[edge, feat].T @ onehot[edge, tgt], where
  onehot[e, t] = (t == col_in_window[e]) * rsqrt(deg[col[e]]) is built by a
  single DVE tensor_scalar op.  Self-loops are ordinary edges.
- Normalization: table rows are pre-scaled by rsqrt(deg[src]); the target
  factor rides inside the one-hot.  deg is an integer histogram of the
  (index-only) edge list, computed host-side; all float math is on-device.
- Layer 1 table is built redundantly on every core (cheaper than a second
  collective); layer 2 table is AllGathered from per-core shards.
"""

import numpy as np

N_REAL = 50000
E_REAL = 800000
D = 128
NCORES = 8
NO_AG = False
ALL_F32 = False
PHASES = {"A", "G1", "S", "G2", "H"}

_CFG_FULL = dict(n=N_REAL, nsh=6250, win=256, split=32768)


def _derive(cfg):
    nsh = cfg["nsh"]
    nloc = ((nsh + 127) // 128) * 128
    win = cfg["win"]
    nloc = ((nloc + win - 1) // win) * win
    npad = NCORES * nloc
    return nloc, npad, nloc // win, npad // 128


def prep(edge_index, cfg=_CFG_FULL):
    """Host-side (integer-only) graph preprocessing -> per-core arrays."""
    n, nsh, win, split = cfg["n"], cfg["nsh"], cfg["win"], cfg["split"]
    nloc, npad, nwin, _ = _derive(cfg)
    row = np.asarray(edge_index[0]).astype(np.int64)
    col = np.asarray(edge_index[1]).astype(np.int64)

    deg = np.bincount(col, minlength=n).astype(np.float32) + 1.0  # + self loop
    deg_t = np.ones(npad, np.float32)
    rr = np.arange(n, dtype=np.int64)
    t_of_r = (rr // nsh) * nloc + (rr % nsh)
    deg_t[t_of_r] = deg

    trow = (row // nsh) * nloc + (row % nsh)
    tcol = (col // nsh) * nloc + (col % nsh)
    core_of = col // nsh

    # per (core, window, class) group sizes -> global NBL/NBH
    per_core = []
    for c in range(NCORES):
        m = core_of == c
        er = trow[m]
        ecl = tcol[m] - c * nloc
        sl = c * nloc + np.arange(nsh, dtype=np.int64)  # self loops
        er = np.concatenate([er, sl])
        ecl = np.concatenate([ecl, np.arange(nsh, dtype=np.int64)])
        w = ecl // win
        is_hi = (er >= split).astype(np.int64)
        key = w * 2 + is_hi
        order = np.argsort(key, kind="stable")
        er, ecl, key = er[order], ecl[order], key[order]
        bounds = np.searchsorted(key, np.arange(2 * nwin + 1))
        per_core.append((er, ecl, bounds))

    nb = np.zeros((NCORES, nwin, 2), np.int64)
    for c in range(NCORES):
        _, _, bounds = per_core[c]
        for w in range(nwin):
            nb[c, w, 0] = bounds[2 * w + 1] - bounds[2 * w]
            nb[c, w, 1] = bounds[2 * w + 2] - bounds[2 * w + 1]
    nbl = int((nb[:, :, 0].max() + 127) // 128)
    nbh = int(max(1, (nb[:, :, 1].max() + 127) // 128))

    cores = []
    for c in range(NCORES):
        er, ecl, bounds = per_core[c]
        arrs = {}
        for cls, nbx in ((0, nbl), (1, nbh)):
            cap = nbx * 128
            src = np.zeros((nwin, cap), np.int64)
            cw = np.full((nwin, cap), -1.0, np.float32)
            dcol = np.ones((nwin, cap), np.float32)
            for w in range(nwin):
                a, b = bounds[2 * w + cls], bounds[2 * w + cls + 1]
                k = b - a
                s = er[a:b] - (split if cls else 0)
                src[w, :k] = s
                cw[w, :k] = (ecl[a:b] % win).astype(np.float32)
                dcol[w, :k] = deg_t[c * nloc + ecl[a:b]]
            # gather idx layout: edge e -> part e%16 (replicated x8), col e//16
            g16 = src.reshape(nwin, cap // 16, 16).transpose(0, 2, 1)  # [w,16,cap/16]
            gidx = np.tile(g16, (1, 8, 1)).transpose(1, 0, 2).reshape(128, nwin * cap // 16)
            # colw/dcol layout: edge e of block b -> part e%128, col w*nbx+b
            cwt = cw.reshape(nwin * nbx, 128).T.copy()
            dct = dcol.reshape(nwin * nbx, 128).T.copy()
            sfx = "lo" if cls == 0 else "hi"
            arrs[f"gidx_{sfx}"] = gidx.astype(np.int16)
            arrs[f"colw_{sfx}"] = cwt
            arrs[f"dcol_{sfx}"] = dct
        # deg of this core's own table rows, [128, nloc/128] tiled
        arrs["degloc"] = deg_t[c * nloc:(c + 1) * nloc].reshape(-1, 128).T.copy()
        cores.append(arrs)

    degt_t = deg_t.reshape(-1, 128).T.copy()  # [128, npad/128]
    return cores, degt_t, nbl, nbh, t_of_r


def build_nc(nbl, nbh, cfg=_CFG_FULL):
    import concourse.bacc as bacc
    import concourse.tile as tile
    import concourse.mybir as mybir
    from concourse.alu_op_type import AluOpType

    nloc, npad, nwin, ntile = _derive(cfg)
    split = cfg["split"]
    f32 = mybir.dt.float32
    f32r = mybir.dt.float32 if ALL_F32 else mybir.dt.float32r
    AF = mybir.ActivationFunctionType
    nsh_t = nloc // 128  # local tiles
    XCH = 16  # stage-A xT chunk, in 128-col tiles

    nc = bacc.Bacc("TRN2", target_bir_lowering=False, debug=False,
                   num_devices=NCORES)
    inp = {}

    def I(name, shape, dt=f32):
        inp[name] = nc.dram_tensor(name, list(shape), dt, kind="ExternalInput").ap()
        return inp[name]

    xT = I("xT", [128, npad])
    W1 = I("W1", [128, 128]); W2 = I("W2", [128, 128]); Wh = I("Wh", [128, 3])
    b1 = I("b1", [128, 1]); b2 = I("b2", [128, 1]); bh = I("bh", [128, 3])
    degt = I("degt", [128, ntile]); degloc = I("degloc", [128, nsh_t])
    iota = I("iota", [128, cfg["win"]])
    g_lo = I("gidx_lo", [128, nwin * nbl * 8], mybir.dt.int16)
    g_hi = I("gidx_hi", [128, nwin * nbh * 8], mybir.dt.int16)
    c_lo = I("colw_lo", [128, nwin * nbl]); c_hi = I("colw_hi", [128, nwin * nbh])
    d_lo = I("dcol_lo", [128, nwin * nbl]); d_hi = I("dcol_hi", [128, nwin * nbh])
    out = nc.dram_tensor("out", [128, nsh_t * 3], f32, kind="ExternalOutput").ap()

    T1 = nc.dram_tensor("T1", [npad, 128], f32r, kind="Internal").ap()
    bounce = nc.dram_tensor("bounce", [nloc, 128], f32r, kind="Internal").ap()
    T2 = nc.dram_tensor("T2", [npad, 128], f32r, kind="Internal",
                        addr_space=("Local" if NO_AG else "Shared")).ap()

    with tile.TileContext(nc) as tc:
        with (
            tc.tile_pool(name="const", bufs=1) as pc,
            tc.tile_pool(name="xch", bufs=2) as pxch,
            tc.tile_pool(name="ha", bufs=3) as pha,
            tc.tile_pool(name="glo", bufs=2) as pglo,
            tc.tile_pool(name="ghi", bufs=2) as pghi,
            tc.tile_pool(name="oh", bufs=6) as poh,
            tc.tile_pool(name="act", bufs=1) as pact,
            tc.tile_pool(name="psA", bufs=2, space="PSUM") as psA,
            tc.tile_pool(name="psW", bufs=2, space="PSUM") as psW,
            tc.tile_pool(name="psH", bufs=2, space="PSUM") as psH,
        ):
            def load(ap, shape, tag, dt=f32):
                t = pc.tile(shape, dt, tag=tag)
                nc.sync.dma_start(t[:], ap[:])
                return t

            iota_sb = load(iota, [128, cfg["win"]], "iota")
            W1_sb = load(W1, [128, 128], "W1"); W2_sb = load(W2, [128, 128], "W2")
            Wh_sb = load(Wh, [128, 3], "Wh")
            b1_sb = load(b1, [128, 1], "b1"); b2_sb = load(b2, [128, 1], "b2")
            bh_sb = load(bh, [128, 3], "bh")
            glo_sb = load(g_lo, [128, nwin * nbl * 8], "glosb", mybir.dt.int16)
            ghi_sb = load(g_hi, [128, nwin * nbh * 8], "ghisb", mybir.dt.int16)
            clo_sb = load(c_lo, [128, nwin * nbl], "closb")
            chi_sb = load(c_hi, [128, nwin * nbh], "chisb")

            def rsqrt_of(ap, cols, tag):
                dsb = load(ap, [128, cols], tag + "_d")
                rec = pc.tile([128, cols], f32, tag=tag + "_r")
                nc.vector.reciprocal(rec[:], dsb[:])
                o = pc.tile([128, cols], f32, tag=tag + "_o")
                nc.scalar.activation(o[:], rec[:], AF.Sqrt)
                return o

            dis_sb = rsqrt_of(degt, ntile, "dis")
            disloc_sb = rsqrt_of(degloc, nsh_t, "disl")
            slo_sb = rsqrt_of(d_lo, nwin * nbl, "slo")
            shi_sb = rsqrt_of(d_hi, nwin * nbh, "shi")

            # persistent activations (feature-major)
            x2T = pact.tile([128, nloc], f32, tag="x2T")
            x3T = pact.tile([128, nloc], f32, tag="x3T")
            out_sb = pact.tile([128, nsh_t * 3], f32, tag="osb")
            nc.vector.memset(x2T[:], 0.0)
            nc.vector.memset(x3T[:], 0.0)

            # ---- stage A: full layer-1 table on every core ----
            for t in range(ntile if "A" in PHASES else 0):
                if t % XCH == 0:
                    xc = pxch.tile([128, XCH * 128], f32, tag="xch")
                    hi = min(npad, (t + XCH) * 128)
                    nc.sync.dma_start(xc[:, : hi - t * 128], xT[:, t * 128: hi])
                ps = psA.tile([128, 128], f32, tag="psA")
                nc.tensor.matmul(ps[:], xc[:, (t % XCH) * 128:(t % XCH + 1) * 128],
                                 W1_sb[:], start=True, stop=True)
                h = pha.tile([128, 128], f32r, tag="ha")
                nc.vector.tensor_scalar(h[:], ps[:], dis_sb[:, t:t + 1], None,
                                        AluOpType.mult)
                nc.sync.dma_start(T1[t * 128:(t + 1) * 128, :], h[:])

            # ---- one GCN aggregation layer ----
            GCH = 8  # max 1024 descriptors per dma_gather call

            def agg_layer(T, xTnext, bias_sb):
                for w in range(nwin):
                    parts = []
                    for cls, pl, nbx, lim, gsb, csb, ssb in (
                            (0, pglo, nbl, (0, split), glo_sb, clo_sb, slo_sb),
                            (1, pghi, nbh, (split, npad), ghi_sb, chi_sb, shi_sb)):
                        for s0 in range(0, nbx, GCH):
                            cs = min(GCH, nbx - s0)
                            gt = pl.tile([128, cs, 128], f32r, tag=f"g{cls}_{s0}")
                            o0 = (w * nbx + s0) * 8
                            nc.gpsimd.dma_gather(
                                gt[:], T[lim[0]:lim[1], :], gsb[:, o0:o0 + cs * 8],
                                num_idxs=cs * 128, num_idxs_reg=cs * 128,
                                elem_size=128)
                            for b in range(cs):
                                parts.append((gt, b, w * nbx + s0 + b, csb, ssb))
                    acc = psW.tile([128, cfg["win"]], f32, tag="acc")
                    for k, (gt, b, B, csb, ssb) in enumerate(parts):
                        oh = poh.tile([128, cfg["win"]], f32r, tag="oh")
                        nc.vector.tensor_scalar(
                            oh[:], iota_sb[:], csb[:, B:B + 1], ssb[:, B:B + 1],
                            AluOpType.is_equal, AluOpType.mult)
                        nc.tensor.matmul(acc[:], gt[:, b, :], oh[:],
                                         start=(k == 0), stop=(k == len(parts) - 1))
                    nc.scalar.activation(xTnext[:, w * cfg["win"]:(w + 1) * cfg["win"]],
                                         acc[:], AF.Relu, bias=bias_sb[:, 0:1])

            if "G1" in PHASES:
                agg_layer(T1, x2T, b1_sb)

            # ---- layer-2 table: local shard + AllGather ----
            for t in range(nsh_t if "S" in PHASES else 0):
                ps = psA.tile([128, 128], f32, tag="psA")
                nc.tensor.matmul(ps[:], x2T[:, t * 128:(t + 1) * 128], W2_sb[:],
                                 start=True, stop=True)
                h = pha.tile([128, 128], f32r, tag="ha")
                nc.vector.tensor_scalar(h[:], ps[:], disloc_sb[:, t:t + 1], None,
                                        AluOpType.mult)
                nc.sync.dma_start(bounce[t * 128:(t + 1) * 128, :], h[:])
            if NO_AG or "S" not in PHASES:
                for t in range(nsh_t if "S" in PHASES else 0):
                    h = pha.tile([128, 128], f32r, tag="ha")
                    nc.sync.dma_start(h[:], bounce[t * 128:(t + 1) * 128, :])
                    nc.sync.dma_start(T2[t * 128:(t + 1) * 128, :], h[:])
            else:
                nc.gpsimd.collective_compute(
                    "AllGather", mybir.AluOpType.bypass,
                    replica_groups=[list(range(NCORES))],
                    ins=[bounce[:]], outs=[T2[:]])

            if "G2" in PHASES:
                agg_layer(T2, x3T, b2_sb)

            # ---- head ----
            for t in range(nsh_t):
                ps = psH.tile([128, 3], f32, tag="psH")
                nc.tensor.matmul(ps[:], x3T[:, t * 128:(t + 1) * 128], Wh_sb[:],
                                 start=True, stop=True)
                nc.vector.tensor_tensor(out_sb[:, t * 3:(t + 1) * 3], ps[:], bh_sb[:],
                                        AluOpType.add)
            nc.sync.dma_start(out[:], out_sb[:])

    nc.compile()
    return nc


def kernel(x, edge_index, W1, b1, W2, b2, Wh, bh, cfg=_CFG_FULL, _trace=False):
    from concourse.bass_utils import run_bass_kernel_spmd

    x = np.asarray(x, dtype=np.float32)
    W1 = np.asarray(W1, np.float32); b1 = np.asarray(b1, np.float32)
    W2 = np.asarray(W2, np.float32); b2 = np.asarray(b2, np.float32)
    Wh = np.asarray(Wh, np.float32); bh = np.asarray(bh, np.float32)
    n, nsh, win = cfg["n"], cfg["nsh"], cfg["win"]
    nloc, npad, nwin, ntile = _derive(cfg)

    cores, degt_t, nbl, nbh, t_of_r = prep(edge_index, cfg)
    nc = build_nc(nbl, nbh, cfg)

    xTp = np.zeros((128, npad), np.float32)
    xTp[:, t_of_r] = x.T  # table-order, feature-major
    iota_np = np.tile(np.arange(win, dtype=np.float32), (128, 1))
    shared = dict(
        xT=xTp, W1=W1, W2=W2, Wh=Wh,
        b1=b1.reshape(128, 1), b2=b2.reshape(128, 1),
        bh=np.tile(bh.reshape(1, 3), (128, 1)).copy(),
        degt=degt_t, iota=iota_np,
    )
    in_maps = [dict(shared, **cores[c]) for c in range(NCORES)]
    res = run_bass_kernel_spmd(nc, in_maps, core_ids=list(range(NCORES)),
                               trace=_trace)

    outs = []
    for c in range(NCORES):
        o = res.results[c]["out"].reshape(128, nloc // 128, 3)
        outs.append(o.transpose(1, 0, 2).reshape(nloc, 3)[:nsh])
    full = np.concatenate(outs, axis=0)[:n]
    if _trace:
        kernel.last_exec_ns = res.exec_time_ns
        kernel.last_trace = (res.instructions_and_trace or (None, None))[1]
    return full



# revision 4
# speedup vs baseline: 1.1611x; 1.1611x over previous
"""GCN (2x GCNConv + linear head) on 8 TRN2 NeuronCores — v2.

Strategy (graph-parallel by target node):
- Nodes sharded across 8 cores (6250 real + pad = 6400 rows/core; table
  row = core*6400 + local).  Layer tables are bf16 in DRAM, rows
  pre-scaled by rsqrt(deg[src]); the target factor is applied once per
  256-target window after aggregation (it distributes out of the sum).
- Edges grouped per (core, window, lo/hi source class); each 128-edge
  block is one bf16 matmul PSUM[feat, tgt] += gathered[edge, feat].T @
  onehot[edge, tgt].  One-hots are pure 0/1, built 8 blocks per DVE
  instruction with stride-0 (broadcast) access patterns.
- Self-loops are not gathered: each window chains 2 extra matmuls whose
  stationary data is the core's own table rows (kept in SBUF) and whose
  moving operand is a constant diagonal one-hot.
- dma_gather cost is pure Q7 descriptor generation (~7.4 ns/idx,
  <=1024 idx/call); block counts are exact per (window, class) maxed
  over cores only (SPMD needs one program).
- Layer-1 table built redundantly on every core; layer-2 table shard is
  AllGathered.  Both layers share the same gather-index/one-hot arrays.
"""

import numpy as np

N_REAL = 50000
E_REAL = 800000
D = 128
NCORES = 8
NSH = 6250
NLOC = 6400
WIN = 256
NWIN = NLOC // WIN          # 25
NPAD = NCORES * NLOC        # 51200
SPLIT = 32768
NTILE = NPAD // 128         # 400
NSH_T = NLOC // 128         # 50
GCH = 8                     # blocks per gather call (1024 idxs max)
XCH = 16


def prep(edge_index):
    """Host-side (integer-only) graph preprocessing -> per-core arrays."""
    row = np.asarray(edge_index[0]).astype(np.int64)
    col = np.asarray(edge_index[1]).astype(np.int64)

    deg = np.bincount(col, minlength=N_REAL).astype(np.float32) + 1.0
    deg_t = np.ones(NPAD, np.float32)
    rr = np.arange(N_REAL, dtype=np.int64)
    t_of_r = (rr // NSH) * NLOC + (rr % NSH)
    deg_t[t_of_r] = deg

    trow = (row // NSH) * NLOC + (row % NSH)
    tcol = (col // NSH) * NLOC + (col % NSH)
    core_of = col // NSH

    per_core = []
    for c in range(NCORES):
        m = core_of == c
        er = trow[m]
        ecl = tcol[m] - c * NLOC
        w = ecl // WIN
        is_hi = (er >= SPLIT).astype(np.int64)
        key = w * 2 + is_hi
        order = np.argsort(key, kind="stable")
        er, ecl, key = er[order], ecl[order], key[order]
        bounds = np.searchsorted(key, np.arange(2 * NWIN + 1))
        per_core.append((er, ecl, bounds))

    # per (window, class) block counts, maxed over cores (one SPMD program)
    nb = np.zeros((NWIN, 2), np.int64)
    for c in range(NCORES):
        _, _, bounds = per_core[c]
        for w in range(NWIN):
            for cls in range(2):
                cnt = bounds[2 * w + cls + 1] - bounds[2 * w + cls]
                nb[w, cls] = max(nb[w, cls], (cnt + 127) // 128)
    nb = np.maximum(nb, 1)
    nblk = int(nb.sum())

    import ml_dtypes
    cores = []
    for c in range(NCORES):
        er, ecl, bounds = per_core[c]
        idx = np.zeros((nblk, 128), np.int64)
        cw = np.full((nblk, 128), -1.0, np.float32)
        B = 0
        for w in range(NWIN):
            for cls in range(2):
                a, b = bounds[2 * w + cls], bounds[2 * w + cls + 1]
                k = b - a
                nbx = int(nb[w, cls])
                s = er[a:b] - (SPLIT if cls else 0)
                fl_i = np.zeros(nbx * 128, np.int64)
                fl_c = np.full(nbx * 128, -1.0, np.float32)
                fl_i[:k] = s
                fl_c[:k] = (ecl[a:b] % WIN).astype(np.float32)
                idx[B:B + nbx] = fl_i.reshape(nbx, 128)
                cw[B:B + nbx] = fl_c.reshape(nbx, 128)
                B += nbx
        # gather idx layout: block b edge e -> idx16[(e%16 wrapped x8), b*8+e//16]
        g16 = idx.reshape(nblk, 8, 16).transpose(2, 0, 1).reshape(16, nblk * 8)
        gidx = np.tile(g16, (8, 1)).astype(np.int16)
        cores.append(dict(
            gidx=gidx,
            cwT=cw.T.copy().astype(ml_dtypes.bfloat16),   # [128, nblk]
        ))

    degt_t = deg_t.reshape(-1, 128).T.copy()              # [128, 400]
    degloc = [deg_t[c * NLOC:(c + 1) * NLOC].reshape(-1, 128).T.copy()
              for c in range(NCORES)]                     # [128, 50] each
    degrow = [deg_t[c * NLOC:(c + 1) * NLOC].reshape(1, NLOC).copy()
              for c in range(NCORES)]                     # [1, 6400] each
    return cores, degt_t, degloc, degrow, nb, nblk, t_of_r


def build_nc(nb, nblk):
    import concourse.bacc as bacc
    import concourse.tile as tile
    import concourse.mybir as mybir
    from concourse.alu_op_type import AluOpType

    f32 = mybir.dt.float32
    f32r = mybir.dt.float32r
    bf16 = mybir.dt.bfloat16
    AF = mybir.ActivationFunctionType

    nc = bacc.Bacc("TRN2", target_bir_lowering=False, debug=False,
                   num_devices=NCORES)
    inp = {}

    def I(name, shape, dt=f32):
        inp[name] = nc.dram_tensor(name, list(shape), dt, kind="ExternalInput").ap()
        return inp[name]

    xT = I("xT", [128, NPAD])
    xlocT = I("xlocT", [128, NLOC])
    W1 = I("W1", [128, 128]); W2 = I("W2", [128, 128]); Wh = I("Wh", [128, 3])
    b1 = I("b1", [128, 1]); b2 = I("b2", [128, 1]); bh = I("bh", [128, 3])
    degt = I("degt", [128, NTILE]); degloc = I("degloc", [128, NSH_T])
    degrow = I("degrow", [1, NLOC])
    iota = I("iota", [128, WIN], bf16)
    pidx = I("pidx", [128, 1])
    gidx = I("gidx", [128, nblk * 8], mybir.dt.int16)
    cwT = I("cwT", [128, nblk], bf16)
    out = nc.dram_tensor("out", [128, NSH_T * 3], f32, kind="ExternalOutput").ap()

    T1 = nc.dram_tensor("T1", [NPAD, 128], bf16, kind="Internal").ap()
    bounce = nc.dram_tensor("bounce", [NLOC, 128], bf16, kind="Internal").ap()
    T2 = nc.dram_tensor("T2", [NPAD, 128], bf16, kind="Internal",
                        addr_space="Shared").ap()

    # per-(window,class) gather-call layout: list of (B0, cs, cls) chunks
    calls = []
    B = 0
    for w in range(NWIN):
        wcalls = []
        for cls in range(2):
            nbx = int(nb[w, cls])
            for s0 in range(0, nbx, GCH):
                cs = min(GCH, nbx - s0)
                wcalls.append((B + s0, cs, cls))
            B += nbx
        calls.append(wcalls)

    with tile.TileContext(nc) as tc:
        with (
            tc.tile_pool(name="const", bufs=1) as pc,
            tc.tile_pool(name="xch", bufs=2) as pxch,
            tc.tile_pool(name="ha", bufs=3) as pha,
            tc.tile_pool(name="g", bufs=3) as pg,
            tc.tile_pool(name="oh", bufs=3) as poh,
            tc.tile_pool(name="act", bufs=1) as pact,
            tc.tile_pool(name="psA", bufs=2, space="PSUM") as psA,
            tc.tile_pool(name="psW", bufs=4, space="PSUM") as psW,
            tc.tile_pool(name="psH", bufs=2, space="PSUM") as psH,
        ):
            def load(ap, shape, tag, dt=f32):
                t = pc.tile(shape, dt, tag=tag)
                nc.sync.dma_start(t[:], ap[:])
                return t

            iota_sb = load(iota, [128, WIN], "iota", bf16)
            pidx_sb = load(pidx, [128, 1], "pidx")
            W1_sb = load(W1.bitcast(f32r), [128, 128], "W1", f32r)
            Wh_sb = load(Wh, [128, 3], "Wh")
            b1_sb = load(b1, [128, 1], "b1"); b2_sb = load(b2, [128, 1], "b2")
            bh_sb = load(bh, [128, 3], "bh")
            gidx_sb = load(gidx, [128, nblk * 8], "gidx", mybir.dt.int16)
            cwT_sb = load(cwT, [128, nblk], "cwT", bf16)

            # W2 / Wh in bf16 (cast on device)
            W2f_sb = load(W2, [128, 128], "W2f")
            W2_sb = pc.tile([128, 128], bf16, tag="W2b")
            nc.vector.tensor_scalar(W2_sb[:], W2f_sb[:], 1.0, None, AluOpType.mult)
            Whb_sb = pc.tile([128, 3], bf16, tag="Whb")
            nc.vector.tensor_scalar(Whb_sb[:], Wh_sb[:], 1.0, None, AluOpType.mult)

            def rsqrt_of(ap, cols, tag, parts=128):
                dsb = load(ap, [parts, cols], tag + "_d")
                rec = pc.tile([parts, cols], f32, tag=tag + "_r")
                nc.vector.reciprocal(rec[:], dsb[:])
                o = pc.tile([parts, cols], f32, tag=tag + "_o")
                nc.scalar.activation(o[:], rec[:], AF.Sqrt)
                return o

            dis_sb = rsqrt_of(degt, NTILE, "dis")
            disloc_sb = rsqrt_of(degloc, NSH_T, "disl")
            disrow_sb = rsqrt_of(degrow, NLOC, "disr", parts=1)
            disrow_b = pc.tile([1, NLOC], bf16, tag="disrb")
            nc.vector.tensor_scalar(disrow_b[:], disrow_sb[:], 1.0, None,
                                    AluOpType.mult)
            disw = pc.tile([128, NLOC], bf16, tag="disw")
            nc.gpsimd.partition_broadcast(disw[:], disrow_b[:])

            # constant diagonal one-hots for self-loop blocks
            ohd0 = pc.tile([128, WIN], bf16, tag="ohd0")
            nc.vector.tensor_scalar(ohd0[:], iota_sb[:], pidx_sb[:], None,
                                    AluOpType.is_equal)
            ohd1 = pc.tile([128, WIN], bf16, tag="ohd1")
            p128 = pc.tile([128, 1], f32, tag="p128")
            nc.vector.tensor_scalar(p128[:], pidx_sb[:], 128.0, None,
                                    AluOpType.add)
            nc.vector.tensor_scalar(ohd1[:], iota_sb[:], p128[:], None,
                                    AluOpType.is_equal)

            # persistent activations / local table shards
            xAct = pact.tile([128, NLOC], bf16, tag="xAct")
            x2T = xAct
            x3T = xAct
            T1loc = pact.tile([128, NSH_T, 128], bf16, tag="T1loc")
            T2loc = pact.tile([128, NSH_T, 128], bf16, tag="T2loc")
            out_sb = pact.tile([128, NSH_T * 3], f32, tag="osb")

            # ---- mini stage A': local layer-1 table rows -> SBUF ----
            for t in range(NSH_T):
                if t % XCH == 0:
                    xlc = pxch.tile([128, XCH * 128], f32r, tag="xch")
                    hi = min(NLOC, (t + XCH) * 128)
                    nc.sync.dma_start(xlc[:, :hi - t * 128],
                                      xlocT.bitcast(f32r)[:, t * 128:hi])
                ps = psA.tile([128, 128], f32, tag="psA")
                nc.tensor.matmul(ps[:], xlc[:, (t % XCH) * 128:(t % XCH + 1) * 128],
                                 W1_sb[:], start=True, stop=True)
                nc.vector.tensor_scalar(T1loc[:, t, :], ps[:],
                                        disloc_sb[:, t:t + 1], None,
                                        AluOpType.mult)

            # ---- stage A: full layer-1 table -> T1 (every core) ----
            for t in range(NTILE):
                if t % XCH == 0:
                    xc = pxch.tile([128, XCH * 128], f32r, tag="xch")
                    hi = min(NPAD, (t + XCH) * 128)
                    nc.sync.dma_start(xc[:, :hi - t * 128],
                                      xT.bitcast(f32r)[:, t * 128:hi])
                ps = psA.tile([128, 128], f32, tag="psA")
                nc.tensor.matmul(ps[:], xc[:, (t % XCH) * 128:(t % XCH + 1) * 128],
                                 W1_sb[:], start=True, stop=True)
                h = pha.tile([128, 128], bf16, tag="ha")
                nc.vector.tensor_scalar(h[:], ps[:], dis_sb[:, t:t + 1], None,
                                        AluOpType.mult)
                nc.sync.dma_start(T1[t * 128:(t + 1) * 128, :], h[:])

            # ---- one GCN aggregation layer ----
            def agg_layer(T, Tloc, xTnext, bias_sb):
                Tlo = T[0:SPLIT, :]
                Thi = T[SPLIT:NPAD, :]
                for w in range(NWIN):
                    acc = psW.tile([128, WIN], f32, tag="acc")
                    # self-loop blocks first (independent of gathers)
                    nc.tensor.matmul(acc[:], Tloc[:, 2 * w, :], ohd0[:],
                                     start=True, stop=False)
                    nc.tensor.matmul(acc[:], Tloc[:, 2 * w + 1, :], ohd1[:],
                                     start=False, stop=False)
                    wcalls = calls[w]
                    for ci, (B0, cs, cls) in enumerate(wcalls):
                        gt = pg.tile([128, GCH, 128], bf16, tag="g")
                        nc.gpsimd.dma_gather(
                            gt[:, :cs, :], Tlo if cls == 0 else Thi,
                            gidx_sb[:, B0 * 8:(B0 + cs) * 8],
                            num_idxs=cs * 128, num_idxs_reg=cs * 128,
                            elem_size=128)
                        oh = poh.tile([128, GCH, WIN], bf16, tag="oh")
                        src0 = iota_sb[:].unsqueeze(1).broadcast_to([128, cs, WIN])
                        src1 = cwT_sb[:, B0:B0 + cs].unsqueeze(2).broadcast_to(
                            [128, cs, WIN])
                        nc.vector.tensor_tensor(oh[:, :cs, :], src0, src1,
                                                AluOpType.is_equal)
                        last = ci == len(wcalls) - 1
                        for b in range(cs):
                            nc.tensor.matmul(acc[:], gt[:, b, :], oh[:, b, :],
                                             start=False,
                                             stop=last and b == cs - 1)
                    tmp = poh.tile([128, WIN], bf16, tag="tmp")
                    nc.vector.tensor_tensor(tmp[:], acc[:],
                                            disw[:, w * WIN:(w + 1) * WIN],
                                            AluOpType.mult)
                    nc.scalar.activation(xTnext[:, w * WIN:(w + 1) * WIN],
                                         tmp[:], AF.Relu, bias=bias_sb[:, 0:1])

            agg_layer(T1, T1loc, x2T, b1_sb)

            # ---- layer-2 table: local shard + AllGather ----
            for t in range(NSH_T):
                ps = psA.tile([128, 128], f32, tag="psA")
                nc.tensor.matmul(ps[:], x2T[:, t * 128:(t + 1) * 128], W2_sb[:],
                                 start=True, stop=True)
                nc.vector.tensor_scalar(T2loc[:, t, :], ps[:],
                                        disloc_sb[:, t:t + 1], None,
                                        AluOpType.mult)
                nc.sync.dma_start(bounce[t * 128:(t + 1) * 128, :],
                                  T2loc[:, t, :])
            nc.gpsimd.collective_compute(
                "AllGather", mybir.AluOpType.bypass,
                replica_groups=[list(range(NCORES))],
                ins=[bounce[:]], outs=[T2[:]])

            agg_layer(T2, T2loc, x3T, b2_sb)

            # ---- head ----
            for t in range(NSH_T):
                ps = psH.tile([128, 3], f32, tag="psH")
                nc.tensor.matmul(ps[:], x3T[:, t * 128:(t + 1) * 128], Whb_sb[:],
                                 start=True, stop=True)
                nc.vector.tensor_tensor(out_sb[:, t * 3:(t + 1) * 3], ps[:],
                                        bh_sb[:], AluOpType.add)
            nc.sync.dma_start(out[:], out_sb[:])

    nc.compile()
    return nc, inp


def kernel(x, edge_index, W1, b1, W2, b2, Wh, bh, _trace=False, _sim=False):
    from concourse.bass_utils import run_bass_kernel_spmd

    x = np.asarray(x, dtype=np.float32)
    W1 = np.asarray(W1, np.float32); b1 = np.asarray(b1, np.float32)
    W2 = np.asarray(W2, np.float32); b2 = np.asarray(b2, np.float32)
    Wh = np.asarray(Wh, np.float32); bh = np.asarray(bh, np.float32)

    cores, degt_t, degloc, degrow, nb, nblk, t_of_r = prep(edge_index)
    nc, _ = build_nc(nb, nblk)

    import ml_dtypes
    xTp = np.zeros((128, NPAD), np.float32)
    xTp[:, t_of_r] = x.T
    iota_np = np.tile(np.arange(WIN, dtype=np.float32), (128, 1)).astype(
        ml_dtypes.bfloat16)
    pidx_np = np.arange(128, dtype=np.float32).reshape(128, 1)
    shared = dict(
        xT=xTp, W1=W1, W2=W2, Wh=Wh,
        b1=b1.reshape(128, 1), b2=b2.reshape(128, 1),
        bh=np.tile(bh.reshape(1, 3), (128, 1)).copy(),
        degt=degt_t, iota=iota_np, pidx=pidx_np,
    )
    in_maps = []
    for c in range(NCORES):
        m = dict(shared)
        m.update(cores[c])
        m["degloc"] = degloc[c]
        m["degrow"] = degrow[c]
        m["xlocT"] = xTp[:, c * NLOC:(c + 1) * NLOC].copy()
        in_maps.append(m)

    if _sim:
        from concourse.bass_interp import MultiCoreSim
        sim = MultiCoreSim(nc, num_cores=NCORES)
        for c, cs in enumerate(sim.cores.values()):
            for k, v in in_maps[c].items():
                cs.tensor(k)[:] = v
        sim.simulate()
        outs_sim = [np.asarray(cs.tensor("out")) for cs in sim.cores.values()]
        outs = []
        for c in range(NCORES):
            o = outs_sim[c].reshape(128, NSH_T, 3)
            outs.append(o.transpose(1, 0, 2).reshape(NLOC, 3)[:NSH])
        return np.concatenate(outs, axis=0)[:N_REAL]

    res = run_bass_kernel_spmd(nc, in_maps, core_ids=list(range(NCORES)),
                               trace=_trace)
    outs = []
    for c in range(NCORES):
        o = res.results[c]["out"].reshape(128, NSH_T, 3)
        outs.append(o.transpose(1, 0, 2).reshape(NLOC, 3)[:NSH])
    full = np.concatenate(outs, axis=0)[:N_REAL]
    if _trace:
        kernel.last_exec_ns = res.exec_time_ns
        kernel.last_trace = (res.instructions_and_trace or (None, None))[1]
    return full
